# revision 24
# baseline (speedup 1.0000x reference)
"""Trainium2 Bass kernel for nn_Attention_75093208203309 (sparse attention).

Contract: kernel(**inputs) takes FULL unsharded inputs (numpy), returns the
FULL [4096, 1024] float32 output. Internally shards query rows across 8
NeuronCores; k/v are computed locally per-core and all-gathered on-device.

v3 design:
  - Projections q/k/v/qc run as float32r matmuls; q/qc/k/v are quantized to
    fp8e4m3 on the ACT engine (k/v before their all-gathers: 0.5 MB each per
    core instead of 2 MB/1 MB).
  - Softmax support reduction: only entries with attention_mask=1 AND
    learnable_mask=1 AND st=1 reach mask level 2, and level-2 entries
    dominate every row, so the mask pipeline reduces to
    keep = (conn_logit > -bias) * (am*lm)  (am*lm combined host-side, one
    uint8 tensor), and E = exp(S/32) * keep with exact zeros.
  - S and conn logits and E@v run as fp8 DoubleRow matmuls (2 contraction
    rows per PE pass): ~2x PE throughput vs f32r. Row sums via DoubleRow
    matmul with fp8 ones.
  - keep on DVE (reads conn PSUM), exp on ACT (reads S PSUM), the mask
    multiply on Pool (SBUF-only; GPSIMD cannot touch PSUM).
  - DMA instruction count minimized (each dma_start costs ~625 ns on the
    shared HWDGE generator): partition-major host layouts let xt / weight
    halves / kt / v / gathered reads move as single large DMAs each.
"""

import contextlib

import numpy as np
import ml_dtypes  # noqa: F401  (np fp8/bf16 views)

import concourse.bass as bass
import concourse.bacc as bacc
import concourse.mybir as mybir
import concourse.tile as tile
from concourse import bass_utils

f32 = mybir.dt.float32
f32r = mybir.dt.float32r
bf16 = mybir.dt.bfloat16
fp8 = mybir.dt.float8e4
AF = mybir.ActivationFunctionType
ALU = mybir.AluOpType
DR = mybir.MatmulPerfMode.DoubleRow

NCORES = 8
N, D = 4096, 1024
M = N // NCORES          # 512 rows per core
MT = M // 128            # 4 m-tiles
G = N // 128             # 32 k-row tiles
GP = G // 2              # 16 k-row tile pairs (DoubleRow E@v)
DC = D // 128            # 8 contraction tiles
WSCALE = 64.0            # fp8 weight pre-scale (avoids e4m3 subnormals)
RG = [list(range(NCORES))]


def build(bias_val: float, timing_mode: bool = False, repeats: int = 1):
    """timing_mode: single-core variant with zk/zv as ExternalInputs and no
    collectives, for TimelineSim cost-model profiling."""
    nc = bacc.Bacc(None, num_devices=NCORES, debug=False)

    xt = nc.dram_tensor("xt", [128, DC, M], fp8, kind="ExternalInput")
    xnb = nc.dram_tensor("xnb", [MT, 128, D], bf16, kind="ExternalInput")
    wqt = nc.dram_tensor("wqt", [2, 128, DC, 512], fp8, kind="ExternalInput")
    wkt = nc.dram_tensor("wkt", [2, 128, DC, 512], fp8, kind="ExternalInput")
    wvt = nc.dram_tensor("wvt", [2, 128, DC, 512], fp8, kind="ExternalInput")
    cn = nc.dram_tensor("cn", [2, 128, DC, 512], bf16, kind="ExternalInput")
    pcombo = nc.dram_tensor("pcombo", [128, MT + 2 * DC], f32,
                            kind="ExternalInput")
    bcombo_d = nc.dram_tensor("bcombo", [1, 128 + D], bf16,
                              kind="ExternalInput")
    mmh = nc.dram_tensor("mmh", [G, 128, M], mybir.dt.uint8, kind="ExternalInput")
    ones8 = nc.dram_tensor("ones8", [128, 2, 8], mybir.dt.float8e4,
                           kind="ExternalInput")
    out = nc.dram_tensor("out", [MT, 128, D], f32, kind="ExternalOutput")

    with tile.TileContext(nc) as tc, contextlib.ExitStack() as ST:
        pp = ST.enter_context(tc.tile_pool(name="persist", bufs=1))
        dp = ST.enter_context(tc.tile_pool(name="dram", bufs=1, space="DRAM"))

        ones_s = pp.tile([128, 2, 8], fp8, name="ones_s")
        pcf = pp.tile([128, MT + 2 * DC], f32, name="pcf")
        bnd_s = pcf[:, 0:MT]
        bq_s = pcf[:, MT : MT + DC]
        bk_s = pcf[:, MT + DC : MT + 2 * DC]
        bcombo = pp.tile([1, 128 + D], bf16, name="bcombo")
        onesk1 = bcombo[:, 0:128]
        bv_s = bcombo[:, 128 : 128 + D]
        recip_s = pp.tile([128, MT], f32, name="recip_s")
        s1_s = pp.tile([128, MT], f32, name="s1_s")

        def load_persists():
            # emitted AFTER the first xt/weight DMAs: the SP queue and the
            # shared HWDGE generator process DMAs in emission order, and
            # nothing here is needed in the first ~15us.
            nc.sync.dma_start(ones_s[:], ones8.ap())
            nc.sync.dma_start(pcf[:], pcombo.ap())
            nc.sync.dma_start(bcombo[:], bcombo_d.ap())

        if timing_mode:
            zk = nc.dram_tensor("zk", [NCORES, 128, DC, M], fp8,
                                kind="ExternalInput").ap()
            zv = nc.dram_tensor("zv", [NCORES, 128, MT, D], fp8,
                                kind="ExternalInput").ap()

        for _rep in range(repeats):
            kt_loc = dp.tile([128, DC, M], fp8, name=f"kt_loc{_rep}")
            v_loc = dp.tile([128, MT, D], fp8, name=f"v_loc{_rep}")
            if not timing_mode:
                zk = dp.tile([NCORES, 128, DC, M], fp8, name=f"zk{_rep}",
                             addr_space="Shared")
                zv = dp.tile([NCORES, 128, MT, D], fp8, name=f"zv{_rep}",
                             addr_space="Shared")
            Ep = [
                pp.tile([128, 2, M], fp8, tag="Ep", name=f"Ep_{p}_{_rep}", bufs=GP)
                for p in range(GP)
            ]
            # pools whose lifetimes cross phase boundaries, closed manually
            q_stack = contextlib.ExitStack()
            qp = q_stack.enter_context(tc.tile_pool(name="qpool", bufs=1))
            kp = q_stack.enter_context(tc.tile_pool(name="s_kt", bufs=3))
            qt_s = qp.tile([128, DC, M], bf16, name="qt_s")
            qt8 = qp.tile([128, DC, M], fp8, name="qt8")
            qct8 = qp.tile([128, DC, M], fp8, name="qct8")

            ktb_pre = {}

            def load_ktb(j):
                ktb = kp.tile([128, DC, M], fp8, tag="kt", name="ktb")
                nc.sync.dma_start(ktb[:], zk[j])
                ktb_pre[j] = ktb
                return ktb

            # ------------- QKV projections (fp8 DoubleRow; conn bf16) -------------
            with (
                tc.tile_pool(name="qkv_w", bufs=3) as wp,
                tc.tile_pool(name="qkv_x", bufs=1) as xp,
                tc.tile_pool(name="qkv_sb", bufs=2) as sp,
                tc.tile_pool(name="qkv_ps", bufs=8, space="PSUM") as ps1,
            ):
                xt_s = xp.tile([128, DC, M], fp8, name="xt_s")
                # first x/weight tiles load in 4-t chunks so the first
                # matmuls start earlier; later loads stay monolithic
                for c in range(2):
                    nc.sync.dma_start(
                        xt_s[:, 4 * c : 4 * c + 4, :],
                        xt.ap()[:, 4 * c : 4 * c + 4, :],
                    )

                def load_w_half(wdram, half, name, chunks=1, dt=fp8):
                    w_h = wp.tile([128, DC, 512], dt, tag="w", name=f"w_{name}{half}")
                    cs = DC // chunks
                    for c in range(chunks):
                        nc.sync.dma_start(
                            w_h[:, c * cs : (c + 1) * cs, :],
                            wdram.ap()[half][:, c * cs : (c + 1) * cs, :],
                        )
                    return w_h

                def mm_half_dr(w_h, rhs8, psums):
                    # fp8 DoubleRow: weights host-scaled by 64; epilogues
                    # divide by 64 on the ACT engine.
                    for tt in range(DC // 2):
                        for oi in range(4):
                            nc.tensor.matmul(
                                psums[oi][:],
                                w_h[:, 2 * tt : 2 * tt + 2,
                                    oi * 128 : (oi + 1) * 128],
                                rhs8[:, 2 * tt : 2 * tt + 2, :],
                                start=(tt == 0),
                                stop=(tt == DC // 2 - 1),
                                perf_mode=DR,
                            )

                def mm_half(w_h, rhs_tile, psums):
                    for t in range(DC):
                        for oi in range(4):
                            nc.tensor.matmul(
                                psums[oi][:],
                                w_h[:, t, oi * 128 : (oi + 1) * 128],
                                rhs_tile[:, t, :],
                                start=(t == 0),
                                stop=(t == DC - 1),
                            )

                # kT first: it feeds the first all-gather.
                wk_h = [load_w_half(wkt, h, "k", chunks=2 if h == 0 else 1)
                        for h in range(2)]
                if _rep == 0:
                    load_persists()
                kpss = []
                for half in range(2):
                    kps = [
                        ps1.tile([128, M], f32, tag="ps1", name=f"kps{half}{i}")
                        for i in range(4)
                    ]
                    mm_half_dr(wk_h[half], xt_s, kps)
                    kpss.append(kps)
                wv_h = [load_w_half(wvt, h, "v") for h in range(2)]
                wq_h = [load_w_half(wqt, h, "q") for h in range(2)]
                cn_h = [load_w_half(cn, h, "c", dt=bf16) for h in range(2)]
                kt_sb = sp.tile([128, DC, M], fp8, name="kt_sb", bufs=1)
                for half in range(2):
                    for oi in range(4):
                        ot = half * 4 + oi
                        nc.scalar.activation(
                            kt_sb[:, ot, :], kpss[half][oi][:], AF.Identity,
                            bias=pcf[:, MT + DC + ot : MT + DC + ot + 1],
                            scale=1.0 / WSCALE,
                        )
                    nc.sync.dma_start(
                        kt_loc[:, 4 * half : 4 * half + 4, :],
                        kt_sb[:, 4 * half : 4 * half + 4, :],
                    )
                if not timing_mode:
                    nc.gpsimd.collective_compute(
                        "AllGather", ALU.bypass, replica_groups=RG,
                        ins=[kt_loc[:].opt()], outs=[zk[:].opt()],
                    )
                load_ktb(0)

                # v last: its gather only needs to land before the O phase
                vpss = []
                for dh in range(2):
                    vps = [
                        ps1.tile([128, 512], f32, tag="ps1", name=f"vps{dh}{mt}")
                        for mt in range(MT)
                    ]
                    for tt in range(DC // 2):
                        for mt in range(MT):
                            nc.tensor.matmul(
                                vps[mt][:],
                                xt_s[:, 2 * tt : 2 * tt + 2,
                                     mt * 128 : (mt + 1) * 128],
                                wv_h[dh][:, 2 * tt : 2 * tt + 2, :],
                                start=(tt == 0),
                                stop=False,
                                perf_mode=DR,
                            )
                    vpss.append(vps)
                v_sb = sp.tile([128, MT, D], fp8, name="v_sb", bufs=1)
                for dh in range(2):
                    for mt in range(MT):
                        # bv is host-scaled by 64 in bcombo; closes the group
                        nc.tensor.matmul(
                            vpss[dh][mt][:],
                            onesk1,
                            bcombo[:, 128 + dh * 512 : 128 + (dh + 1) * 512],
                            start=False,
                            stop=True,
                        )
                        nc.scalar.activation(
                            v_sb[:, mt, dh * 512 : (dh + 1) * 512],
                            vpss[dh][mt][:], AF.Identity, scale=1.0 / WSCALE,
                        )
                nc.sync.dma_start(v_loc[:], v_sb[:])
                if not timing_mode:
                    nc.gpsimd.collective_compute(
                        "AllGather", ALU.bypass, replica_groups=RG,
                        ins=[v_loc[:].opt()], outs=[zv[:].opt()],
                    )

                # q next: it feeds the conn projection (bf16 moving operand)
                qpss = []
                for half in range(2):
                    qps = [
                        ps1.tile([128, M], f32, tag="ps1", name=f"qps{half}{i}")
                        for i in range(4)
                    ]
                    mm_half_dr(wq_h[half], xt_s, qps)
                    qpss.append(qps)
                for half in range(2):
                    for oi in range(4):
                        ot = half * 4 + oi
                        nc.scalar.activation(
                            qt_s[:, ot, :], qpss[half][oi][:], AF.Identity,
                            bias=pcf[:, MT + ot : MT + ot + 1],
                            scale=1.0 / WSCALE,
                        )
                        # fp8 copy runs on the (idle) DVE, off the ACT path
                        nc.vector.tensor_copy(qt8[:, ot, :], qt_s[:, ot, :])

                for half in range(2):
                    cps = [
                        ps1.tile([128, M], f32, tag="ps1", name=f"cps{half}{i}")
                        for i in range(4)
                    ]
                    mm_half(cn_h[half], qt_s, cps)
                    for oi in range(4):
                        ot = half * 4 + oi
                        nc.scalar.copy(qct8[:, ot, :], cps[oi][:])

            # v tiles + xn survive into the O phase
            o_stack = contextlib.ExitStack()
            vpool = o_stack.enter_context(
                tc.tile_pool(name="o_v", bufs=3, side="right")
            )
            xop = o_stack.enter_context(
                tc.tile_pool(name="o_x", bufs=1, side="right")
            )
            xn_s = xop.tile([128, MT, D], bf16, name="xn_s")
            nc.sync.dma_start(xn_s[:], xnb.ap().rearrange("m p d -> p m d"))
            vt_pre = {}

            def load_vt(dh, j):
                # [128, MT, 512]: v rows for core j's 4 k-blocks, d-half dh
                vt = vpool.tile([128, MT, 512], fp8, tag="v", name="vt", bufs=16)
                nc.sync.dma_start(
                    vt[:], zv[j][:, :, dh * 512 : (dh + 1) * 512]
                )
                vt_pre[(dh, j)] = vt
                return vt

            # ------- fused S + O(dh0) phase: logits, keep, exp, E@v-half0 -------
            # PSUM budget: psA(2) + psB(2) + O_dh0(4) = 8 banks, so the
            # E@v accumulation for d-half 0 proceeds DURING the S loop
            # instead of serializing behind it.
            o_stack2 = contextlib.ExitStack()
            psO0 = o_stack2.enter_context(
                tc.tile_pool(name="o_ps0", bufs=1, space="PSUM"))
            O_ps0 = [
                psO0.tile([128, 512], f32, tag="O0", name=f"O0_{mt}", bufs=4)
                for mt in range(MT)
            ]
            with (
                tc.tile_pool(name="s_m", bufs=4) as mp,
                tc.tile_pool(name="s_t", bufs=4) as tpool,
                tc.tile_pool(name="s_psA", bufs=2, space="PSUM") as psA,
                tc.tile_pool(name="s_psB", bufs=2, space="PSUM") as psB,
            ):
                for j in range(NCORES):
                    load_vt(0, j)
                for j in range(NCORES):
                    ktb = ktb_pre.pop(j, None) or load_ktb(j)
                    ktb_pre.pop(j, None)
                    for pb in range(2):
                        mm_t = mp.tile([128, 2, M], mybir.dt.uint8, tag="mm",
                                       name="mm_t")
                        nc.sync.dma_start(
                            mm_t[:],
                            mmh.ap()[4 * j + 2 * pb : 4 * j + 2 * pb + 2]
                            .rearrange("g p m -> p g m"),
                        )
                        for gi2 in range(2):
                            gi = 2 * pb + gi2
                            g = j * 4 + gi
                            B = psB.tile([128, M], f32, tag="B", name="Bps")
                            A = psA.tile([128, M], f32, tag="A", name="Aps")
                            # interleaved B/A pairs share lhsT
                            for tt in range(DC // 2):
                                lhsT = ktb[:, 2 * tt : 2 * tt + 2,
                                           gi * 128 : (gi + 1) * 128]
                                nc.tensor.matmul(
                                    B[:], lhsT, qt8[:, 2 * tt : 2 * tt + 2, :],
                                    start=(tt == 0), stop=(tt == DC // 2 - 1),
                                    perf_mode=DR,
                                )
                                nc.tensor.matmul(
                                    A[:], lhsT, qct8[:, 2 * tt : 2 * tt + 2, :],
                                    start=(tt == 0), stop=(tt == DC // 2 - 1),
                                    perf_mode=DR,
                                )
                            keep = tpool.tile([128, M], f32, tag="keep", name="keep")
                            nc.vector.scalar_tensor_tensor(
                                keep[:], A[:], -bias_val, mm_t[:, gi2, :],
                                ALU.is_gt, ALU.mult,
                            )
                            e1 = tpool.tile([128, M], bf16, tag="e1", name="e1")
                            nc.scalar.activation(
                                e1[:], B[:], AF.Exp, scale=1.0 / 32.0
                            )
                            # Masked entries become exact 0 in Ep. Pool
                            # (GPSIMD) can't read PSUM, so e1/keep are SBUF;
                            # every 4th tile runs on DVE so neither vector
                            # engine paces the S loop above PE.
                            gp, ep_i = divmod(g, 2)
                            eng = nc.vector if g % 4 == 3 else nc.gpsimd
                            eng.tensor_tensor(
                                Ep[gp][:, ep_i, :], e1[:], keep[:], ALU.mult
                            )
                    if j == 4:
                        # prefetch dh1 v tiles; transfers overlap S compute
                        for j1 in range(NCORES):
                            load_vt(1, j1)
                    # E@v d-half 0 for this j's two pairs, fused into S
                    vt = vt_pre.pop((0, j))
                    for b in range(2):
                        p = 2 * j + b
                        for mt in range(MT):
                            nc.tensor.matmul(
                                O_ps0[mt][:],
                                Ep[p][:, :, mt * 128 : (mt + 1) * 128],
                                vt[:, 2 * b : 2 * b + 2, :],
                                start=(p == 0),
                                stop=(p == GP - 1),
                                perf_mode=DR,
                            )
            q_stack.close()  # qt/qct + ktb SBUF released before dh1 phase

            # drain O(dh0) psum to SBUF so dh1 gets all 8 banks (O1 + sums)
            o0p = o_stack.enter_context(
                tc.tile_pool(name="o0_sb", bufs=1, side="right"))
            O0_sb = []
            for mt in range(MT):
                t0 = o0p.tile([128, 512], f32, name=f"O0sb_{mt}")
                nc.scalar.copy(t0[:], O_ps0[mt][:])
                O0_sb.append(t0)
            o_stack2.close()

            # -------- dh1 phase: E @ v-half1, row sums, blends, stores --------
            with (
                tc.tile_pool(name="o_out", bufs=2) as opool,
                tc.tile_pool(name="o_ps", bufs=1, space="PSUM") as psO,
            ):
                O_ps1 = [
                    psO.tile([128, 512], f32, tag="O1", name=f"O1_{mt}", bufs=4)
                    for mt in range(MT)
                ]
                S_ps = [
                    psO.tile([128, 8], f32, tag="Ssum", name=f"S{mt}", bufs=4)
                    for mt in range(MT)
                ]
                # sums first: Ep is fully materialized, so the row sums
                # finish ~7us before the O1 accumulation does; the dh0
                # blends + store then overlap the O1 matmuls.
                for p in range(GP):
                    for mt in range(MT):
                        nc.tensor.matmul(
                            S_ps[mt][:],
                            Ep[p][:, :, mt * 128 : (mt + 1) * 128],
                            ones_s[:],
                            start=(p == 0),
                            stop=(p == GP - 1),
                            perf_mode=DR,
                        )
                for j in range(NCORES):
                    vt = vt_pre.pop((1, j))
                    for b in range(2):
                        p = 2 * j + b
                        for mt in range(MT):
                            nc.tensor.matmul(
                                O_ps1[mt][:],
                                Ep[p][:, :, mt * 128 : (mt + 1) * 128],
                                vt[:, 2 * b : 2 * b + 2, :],
                                start=(p == 0),
                                stop=(p == GP - 1),
                                perf_mode=DR,
                            )
                ot_st = [
                    opool.tile([128, MT, 512], f32, tag="ot", name=f"ot_st{dh}")
                    for dh in range(2)
                ]
                for mt in range(MT):
                    nc.vector.reciprocal(
                        recip_s[:, mt : mt + 1], S_ps[mt][:, 0:1]
                    )
                    nc.vector.tensor_tensor(
                        s1_s[:, mt : mt + 1],
                        recip_s[:, mt : mt + 1],
                        pcf[:, mt : mt + 1],
                        ALU.mult,
                    )
                    # dh0 blends run as soon as sums land (sums-first),
                    # overlapping the O1 matmuls; store follows.
                    nc.vector.scalar_tensor_tensor(
                        ot_st[0][:, mt, :], O0_sb[mt][:], s1_s[:, mt : mt + 1],
                        xn_s[:, mt, 0:512], ALU.mult, ALU.add,
                    )
                nc.sync.dma_start(
                    out.ap()[:, :, 0:512].rearrange("m p d -> p m d"),
                    ot_st[0][:],
                )
                for mt in range(MT):
                    nc.vector.scalar_tensor_tensor(
                        ot_st[1][:, mt, :], O_ps1[mt][:], s1_s[:, mt : mt + 1],
                        xn_s[:, mt, 512:1024], ALU.mult, ALU.add,
                    )
                nc.sync.dma_start(
                    out.ap()[:, :, 512:1024].rearrange("m p d -> p m d"),
                    ot_st[1][:],
                )
            o_stack.close()


    nc.compile()
    return nc


def make_in_maps(x, attention_mask, learnable_mask, boundary_mask,
                 W_q, b_q, W_k, b_k, W_v, b_v, connection):
    x = np.asarray(x, np.float32)
    mm_full = (np.asarray(attention_mask, np.float32)
               * np.asarray(learnable_mask, np.float32)).astype(np.uint8)
    boundary = np.asarray(boundary_mask, np.float32).reshape(N)

    def w_halves(wt, dt, scale=1.0):  # wt: [D, D], rows = contraction dim
        # -> [2, 128, DC, 512]: [half][p][t][d] = wt[t*128+p][half*512+d]
        a = np.asarray(wt, np.float32).reshape(DC, 128, 2, 512) * scale
        return np.ascontiguousarray(a.transpose(2, 1, 0, 3)).astype(dt)

    wqt_h = w_halves(np.asarray(W_q, np.float32).T, ml_dtypes.float8_e4m3, WSCALE)
    wkt_h = w_halves(np.asarray(W_k, np.float32).T, ml_dtypes.float8_e4m3, WSCALE)
    wvt_h = w_halves(np.asarray(W_v, np.float32).T, ml_dtypes.float8_e4m3, WSCALE)
    cn_h = w_halves(np.asarray(connection, np.float32), ml_dtypes.bfloat16)
    bq_h = np.asarray(b_q, np.float32).reshape(DC, 128).T
    bk_h = np.asarray(b_k, np.float32).reshape(DC, 128).T
    bcombo_h = np.concatenate(
        [np.ones((1, 128), np.float32),
         WSCALE * np.asarray(b_v, np.float32).reshape(1, D)],
        axis=1).astype(ml_dtypes.bfloat16)
    in_maps = []
    for c in range(NCORES):
        rows = slice(c * M, (c + 1) * M)
        xtc = np.ascontiguousarray(    # [128, DC, M]: [p][t][m] = x[m][t*128+p]
            x[rows].T.reshape(DC, 128, M).transpose(1, 0, 2)).astype(
            ml_dtypes.float8_e4m3)
        in_maps.append(dict(
            xt=xtc,
            xnb=np.ascontiguousarray(
                (1.0 - boundary[rows][:, None]) * x[rows]).reshape(
                MT, 128, D).astype(ml_dtypes.bfloat16),
            wqt=wqt_h, wkt=wkt_h, wvt=wvt_h, cn=cn_h,
            pcombo=np.ascontiguousarray(np.concatenate(
                [boundary[rows].reshape(MT, 128).T, bq_h, bk_h], axis=1)),
            bcombo=bcombo_h,
            mmh=np.ascontiguousarray(mm_full[rows].T).reshape(G, 128, M),
            ones8=np.ones((128, 2, 8), dtype=ml_dtypes.float8_e4m3),
        ))
    return in_maps


_cache = {}


def kernel(x, attention_mask, learnable_mask, boundary_mask,
           W_q, b_q, W_k, b_k, W_v, b_v, connection, bias):
    bias_val = float(np.asarray(bias).reshape(-1)[0])
    if bias_val not in _cache:
        _cache[bias_val] = build(bias_val)
    nc = _cache[bias_val]
    in_maps = make_in_maps(x, attention_mask, learnable_mask, boundary_mask,
                           W_q, b_q, W_k, b_k, W_v, b_v, connection)
    res = bass_utils.run_bass_kernel_spmd(nc, in_maps, core_ids=list(range(NCORES)))
    outs = [res.results[c]["out"].reshape(M, D) for c in range(NCORES)]
    return np.concatenate(outs, axis=0).astype(np.float32)
